# revision 8
# baseline (speedup 1.0000x reference)
"""Trainium2 Bass kernel for a LoRA self-attention block (diffusers-style
CustomLoRAAttnProcessor).

  B=8, S=1024, D=1280, H=20 heads x HD=64, LoRA rank 4 (folded into the
  weights on the host: W_eff = W + 0.25 * B @ A, mathematically identical).

Distribution: pure data parallelism — one batch element per NeuronCore
(8 cores), no collectives.

Per-core layout choices (contraction dim always on SBUF partitions; host
pre-transposes x and the effective weights; all matmul operands bf16 —
fp32 matmuls cost 4x on TRN2):

  main loop over head pairs t=0..9: qT/kT projection for pair t+1
            (128-wide weight stripe), v projection in 2-pair chunks
            (token-major with a ones-column per head: the AV matmul then
            also emits the softmax denominator for free), scoresT[k,q]
            for pair t (the two heads row-pack the PE via base_partition
            0/64), exp on ACT (scale=1/8 folded, no max-subtraction:
            |scores| <= ~6), AV + sumexp, normalize (64-lane reciprocal
            via DMA reshape + gpsimd partition broadcast).  The
            fine-grained projection interleave keeps the PE dense while
            ACT grinds through the exps, so ACT finishes well before the
            PE runs out of work.
  phase O : outT = WoT-style matmul over ctxT + bias, DMA out.
"""

import sys

for _p in ("/opt/trn_rl_repo",):
    if _p not in sys.path:
        sys.path.insert(0, _p)

from contextlib import ExitStack

import ml_dtypes
import numpy as np

import concourse.bass as bass  # noqa: F401  (import order: bass before tile)
import concourse.tile as tile
from concourse import bacc, mybir
from concourse.bass_utils import run_bass_kernel_spmd

B, S, D = 8, 1024, 1280
H, HD = 20, 64
SCALING = 0.25  # alpha / rank
ATTN_SCALE = 1.0 / 8.0  # 1/sqrt(HD)

DT = D // 128  # 10 feature tiles == head pairs
KC = S // 128  # 8 key-position chunks
MG = 5  # output-projection column groups of 256 (2 output tiles each)
VW = HD + 1  # v columns per head incl. ones column

F32 = mybir.dt.float32
BF16 = mybir.dt.bfloat16
FP16 = mybir.dt.float16
EXP = mybir.ActivationFunctionType.Exp

N_CORES = 8


def _qk_t(nc, xT_sb, wpool, pp, wdram, dst, t):
    """One 128-wide column stripe of a feature-major projection:
    dst[:, t, :] = (W.T @ x.T)[128t:128(t+1), :]. The weight tile for
    feature chunk kk is reused across both q-column halves."""
    stripe = wpool.tile([128, DT, 128], BF16, tag="w", name=f"w{t}")
    nc.sync.dma_start(
        out=stripe,
        in_=wdram[:, t * 128 : (t + 1) * 128].rearrange("(t p) n -> p t n", p=128),
    )
    ps0 = pp.tile([128, 512], F32, tag="pp", name=f"ps0_{t}")
    ps1 = pp.tile([128, 512], F32, tag="pp", name=f"ps1_{t}")
    for kk in range(DT):
        lhsT = stripe[:, kk, :]
        nc.tensor.matmul(
            ps0, lhsT=lhsT, rhs=xT_sb[:, kk, 0:512],
            start=(kk == 0), stop=(kk == DT - 1),
        )
        nc.tensor.matmul(
            ps1, lhsT=lhsT, rhs=xT_sb[:, kk, 512:1024],
            start=(kk == 0), stop=(kk == DT - 1),
        )
    nc.vector.tensor_copy(out=dst[:, t, 0:512], in_=ps0)
    nc.vector.tensor_copy(out=dst[:, t, 512:1024], in_=ps1)


def _emit(nc, tc, xT, wqT, wkT, wvT, woT, bo, outT):
    persist_cm = tc.tile_pool(name="persist", bufs=1)
    persist = persist_cm.__enter__()
    qT_sb = persist.tile([128, DT, S], BF16)
    kT_sb = persist.tile([128, DT, S], BF16)
    v_sb = persist.tile([128, KC, H * VW], FP16)
    ctxT_sb = persist.tile([128, DT, S], BF16)
    bo_sb = persist.tile([128, DT, 1], F32)
    nc.sync.dma_start(out=bo_sb, in_=bo[:].rearrange("(t p) -> p t", p=128))
    nc.vector.memset(
        v_sb[:].rearrange("p a (h c) -> p a h c", c=VW)[:, :, :, HD : HD + 1], 1.0
    )

    xpool_cm = tc.tile_pool(name="xpool", bufs=1)
    xpool = xpool_cm.__enter__()
    xT_sb = xpool.tile([128, DT, S], BF16)
    for t in range(DT):
        nc.sync.dma_start(out=xT_sb[:, t, :], in_=xT[t * 128 : (t + 1) * 128, :])

    # -------- main loop: per-head-pair projection + attention ----
    # (the v projection is folded into the loop in 2-pair chunks so the
    # PE keeps dense filler while ACT grinds through the exps)
    def _v_chunk(tp):
        """v (token-major) for head pairs tp, tp+1 (4 heads, 256 cols)."""
        stripe = wvpool.tile([128, DT, 256], BF16, tag="wv", name=f"wv{tp}")
        nc.sync.dma_start(
            out=stripe,
            in_=wvT[:, tp * 256 : (tp + 1) * 256].rearrange(
                "(t p) n -> p t n", p=128
            ),
        )
        for st in range(KC):
            vps = pp.tile([128, 256], F32, tag="pp", name=f"vps{tp}_{st}")
            for kk in range(DT):
                nc.tensor.matmul(
                    vps, lhsT=xT_sb[:, kk, st * 128 : (st + 1) * 128],
                    rhs=stripe[:, kk, :],
                    start=(kk == 0), stop=(kk == DT - 1),
                )
            nc.vector.tensor_copy(
                out=v_sb[
                    :, st, (4 * tp) * VW : (4 * tp + 4) * VW
                ].rearrange("p (h c) -> p h c", c=VW)[:, :, 0:HD],
                in_=vps[:].rearrange("p (h c) -> p h c", c=HD),
            )

    with ExitStack() as p2:
        wpool = p2.enter_context(tc.tile_pool(name="wpool", bufs=4))
        wvpool = p2.enter_context(tc.tile_pool(name="wvpool", bufs=2))
        epool = p2.enter_context(tc.tile_pool(name="epool", bufs=3))
        small = p2.enter_context(tc.tile_pool(name="small", bufs=2))
        pp = p2.enter_context(tc.tile_pool(name="pp", bufs=3, space="PSUM"))
        sc = p2.enter_context(tc.tile_pool(name="sc", bufs=2, space="PSUM"))
        cx = p2.enter_context(tc.tile_pool(name="cx", bufs=1, space="PSUM"))

        _qk_t(nc, xT_sb, wpool, pp, wqT, qT_sb, 0)
        _qk_t(nc, xT_sb, wpool, pp, wkT, kT_sb, 0)
        _v_chunk(0)

        for t in range(DT):
            # pair t+1's projection one iteration ahead — except pair 9,
            # which is projected just-in-time inside its own iteration so
            # the final iteration still has dense PE filler under ACT.
            if t + 1 < DT and t != 8:
                _qk_t(nc, xT_sb, wpool, pp, wqT, qT_sb, t + 1)
                _qk_t(nc, xT_sb, wpool, pp, wkT, kT_sb, t + 1)
            if t == 9:
                _qk_t(nc, xT_sb, wpool, pp, wqT, qT_sb, 9)
                _qk_t(nc, xT_sb, wpool, pp, wkT, kT_sb, 9)
            if t % 2 == 1 and t < 8:
                _v_chunk((t + 1) // 2)

            exps = [
                epool.tile([128, KC, S], FP16, tag="exp", name=f"exp{t}_{i}")
                for i in range(2)
            ]
            # scoresT + exp; head pair (2t, 2t+1) row-packs the PE
            for kc in range(KC):
                for half in range(2):
                    p0 = half * 64
                    ps = sc.tile([128, 1024], F32, tag="sc", name="scps")
                    for qc in range(2):
                        nc.tensor.matmul(
                            ps[:, qc * 512 : (qc + 1) * 512],
                            lhsT=kT_sb[p0 : p0 + 64, t, kc * 128 : (kc + 1) * 128],
                            rhs=qT_sb[p0 : p0 + 64, t, qc * 512 : (qc + 1) * 512],
                            start=True,
                            stop=True,
                        )
                    nc.vector.tensor_copy(out=exps[half][:, kc, :], in_=ps)
            # one big in-place exp per half: amortizes the ~352-cycle
            # ACTIVATE overhead across all 8 kc chunks (fp16 keeps the
            # scaled scores at ~2^-11 relative error)
            for half in range(2):
                nc.scalar.activation(
                    out=exps[half][:],
                    in_=exps[half][:],
                    func=EXP,
                    scale=ATTN_SCALE,
                )
            # AV + sumexp + normalize
            for half in range(2):
                h = 2 * t + half
                stage = small.tile([HD + 1, S], F32, tag="stage", name="stage")
                for qc in range(2):
                    cps = cx.tile([HD + 1, 512], F32, tag="cx", name="cxps")
                    for kc in range(KC):
                        nc.tensor.matmul(
                            cps,
                            lhsT=v_sb[:, kc, h * VW : (h + 1) * VW],
                            rhs=exps[half][:, kc, qc * 512 : (qc + 1) * 512],
                            start=(kc == 0),
                            stop=(kc == KC - 1),
                        )
                    nc.vector.tensor_copy(
                        out=stage[:, qc * 512 : (qc + 1) * 512], in_=cps
                    )
                # 1/sumexp: reshape the [1, S] row across 64 DVE lanes
                r64 = small.tile([64, 16], F32, tag="r64", name="r64")
                nc.sync.dma_start(out=r64, in_=stage[HD : HD + 1, :])
                rc64 = small.tile([64, 16], F32, tag="rc64", name="rc64")
                nc.vector.reciprocal(rc64, r64)
                rrow = small.tile([1, S], F32, tag="rrow", name="rrow")
                nc.sync.dma_start(out=rrow, in_=rc64)
                bcast = small.tile([HD, S], F32, tag="bcast", name="bcast")
                nc.gpsimd.partition_broadcast(bcast, rrow)
                nc.vector.tensor_mul(
                    ctxT_sb[half * 64 : half * 64 + 64, t, :],
                    stage[0:HD, :],
                    bcast,
                )

    xpool_cm.__exit__(None, None, None)

    # ---------------- phase O: output projection ----------------
    with ExitStack() as p3:
        wpool3 = p3.enter_context(tc.tile_pool(name="wpool3", bufs=2))
        opp = p3.enter_context(tc.tile_pool(name="opp", bufs=4, space="PSUM"))
        ostage = p3.enter_context(tc.tile_pool(name="ostage", bufs=4))
        for mg in range(MG):
            stripe = wpool3.tile([128, DT, 256], BF16, tag="w3", name=f"w3_{mg}")
            nc.sync.dma_start(
                out=stripe,
                in_=woT[:, mg * 256 : (mg + 1) * 256].rearrange(
                    "(t p) n -> p t n", p=128
                ),
            )
            for ml in range(2):
                m = mg * 2 + ml
                for qc in range(2):
                    ps = opp.tile([128, 512], F32, tag="opp", name="opps")
                    for kk in range(DT):
                        nc.tensor.matmul(
                            ps,
                            lhsT=stripe[:, kk, ml * 128 : (ml + 1) * 128],
                            rhs=ctxT_sb[:, kk, qc * 512 : (qc + 1) * 512],
                            start=(kk == 0),
                            stop=(kk == DT - 1),
                        )
                    o_sb = ostage.tile([128, 512], F32, tag="ostage", name="osb")
                    nc.vector.tensor_scalar_add(o_sb, ps, bo_sb[:, m, :])
                    nc.sync.dma_start(
                        out=outT[m * 128 : (m + 1) * 128, qc * 512 : (qc + 1) * 512],
                        in_=o_sb,
                    )
    persist_cm.__exit__(None, None, None)


def build_nc():
    nc = bacc.Bacc(None, target_bir_lowering=False)
    xT = nc.dram_tensor("xT", [D, S], BF16, kind="ExternalInput")
    wqT = nc.dram_tensor("wqT", [D, D], BF16, kind="ExternalInput")
    wkT = nc.dram_tensor("wkT", [D, D], BF16, kind="ExternalInput")
    wvT = nc.dram_tensor("wvT", [D, D], BF16, kind="ExternalInput")
    woT = nc.dram_tensor("woT", [D, D], BF16, kind="ExternalInput")
    bo = nc.dram_tensor("bo", [D], F32, kind="ExternalInput")
    outT = nc.dram_tensor("outT", [D, S], F32, kind="ExternalOutput")
    with tile.TileContext(nc) as tc:
        _emit(nc, tc, xT, wqT, wkT, wvT, woT, bo, outT)
    nc.compile()
    return nc


_NC = None


def _get_nc():
    global _NC
    if _NC is None:
        _NC = build_nc()
    return _NC


def make_in_maps(hidden_states, Wq, Wk, Wv, Wo, bo, Aq, Bq, Ak, Bk, Av, Bv, Ao, Bo):
    x = np.asarray(hidden_states, dtype=np.float32)

    def eff_T(W, A, Bup):
        W64 = np.asarray(W, dtype=np.float64)
        lora = np.asarray(Bup, dtype=np.float64) @ np.asarray(A, dtype=np.float64)
        return np.ascontiguousarray(
            (W64 + SCALING * lora).T.astype(ml_dtypes.bfloat16)
        )

    base = {
        "wqT": eff_T(Wq, Aq, Bq),
        "wkT": eff_T(Wk, Ak, Bk),
        "wvT": eff_T(Wv, Av, Bv),
        "woT": eff_T(Wo, Ao, Bo),
        "bo": np.ascontiguousarray(np.asarray(bo, dtype=np.float32)),
    }
    return [
        dict(base, xT=np.ascontiguousarray(x[b].T.astype(ml_dtypes.bfloat16)))
        for b in range(x.shape[0])
    ]


def kernel(**inputs):
    in_maps = make_in_maps(**inputs)
    nc = _get_nc()
    res = run_bass_kernel_spmd(nc, in_maps, core_ids=list(range(N_CORES)))
    out = np.stack([res.results[b]["outT"].T for b in range(N_CORES)])
    return np.ascontiguousarray(out, dtype=np.float32)


# revision 12
# speedup vs baseline: 1.2376x; 1.2376x over previous
"""Trainium2 Bass kernel for a LoRA self-attention block (diffusers-style
CustomLoRAAttnProcessor).

  B=8, S=1024, D=1280, H=20 heads x HD=64, LoRA rank 4 (folded into the
  weights on the host: W_eff = W + 0.25 * B @ A, mathematically identical).

Distribution: pure data parallelism — one batch element per NeuronCore
(8 cores), no collectives.

Per-core layout choices (contraction dim always on SBUF partitions; host
pre-transposes x and the effective weights; all matmul operands bf16 —
fp32 matmuls cost 4x on TRN2):

  main loop over head pairs t=0..9: qT/kT projection for pair t+1
            (128-wide weight stripe), v projection in 2-pair chunks
            (token-major with a ones-column per head: the AV matmul then
            also emits the softmax denominator for free), scoresT[k,q]
            for pair t (the two heads row-pack the PE via base_partition
            0/64), exp on ACT (scale=1/8 folded, no max-subtraction:
            |scores| <= ~6), AV + sumexp, normalize (64-lane reciprocal
            via DMA reshape + gpsimd partition broadcast).  The
            fine-grained projection interleave keeps the PE dense while
            ACT grinds through the exps, so ACT finishes well before the
            PE runs out of work.
  phase O : outT = WoT-style matmul over ctxT + bias, DMA out.
"""

import sys

for _p in ("/opt/trn_rl_repo",):
    if _p not in sys.path:
        sys.path.insert(0, _p)

from contextlib import ExitStack

import ml_dtypes
import numpy as np

import concourse.bass as bass  # noqa: F401  (import order: bass before tile)
import concourse.tile as tile
from concourse import bacc, mybir
from concourse.bass_utils import run_bass_kernel_spmd

B, S, D = 8, 1024, 1280
H, HD = 20, 64
SCALING = 0.25  # alpha / rank
ATTN_SCALE = 1.0 / 8.0  # 1/sqrt(HD)

DT = D // 128  # 10 feature tiles == head pairs
KC = S // 128  # 8 key-position chunks
MG = 5  # output-projection column groups of 256 (2 output tiles each)
VW = HD + 1  # v columns per head incl. ones column

F32 = mybir.dt.float32
BF16 = mybir.dt.bfloat16
FP16 = mybir.dt.float16
EXP = mybir.ActivationFunctionType.Exp

N_CORES = 8


def _qk_t(nc, xT_sb, wpool, pp, wdram, dst, t):
    """One 128-wide column stripe of a feature-major projection:
    dst[:, t, :] = (W.T @ x.T)[128t:128(t+1), :]. The weight tile for
    feature chunk kk is reused across both q-column halves."""
    stripe = wpool.tile([128, DT, 128], BF16, tag="w", name=f"w{t}")
    nc.sync.dma_start(
        out=stripe,
        in_=wdram[:, t * 128 : (t + 1) * 128].rearrange("(t p) n -> p t n", p=128),
    )
    ps0 = pp.tile([128, 512], F32, tag="pp", name=f"ps0_{t}")
    ps1 = pp.tile([128, 512], F32, tag="pp", name=f"ps1_{t}")
    for kk in range(DT):
        lhsT = stripe[:, kk, :]
        nc.tensor.matmul(
            ps0, lhsT=lhsT, rhs=xT_sb[:, kk, 0:512],
            start=(kk == 0), stop=(kk == DT - 1),
        )
        nc.tensor.matmul(
            ps1, lhsT=lhsT, rhs=xT_sb[:, kk, 512:1024],
            start=(kk == 0), stop=(kk == DT - 1),
        )
    nc.vector.tensor_copy(out=dst[:, t, 0:512], in_=ps0)
    nc.vector.tensor_copy(out=dst[:, t, 512:1024], in_=ps1)


def _emit(nc, tc, xT, wqT, wkT, wvT, woT, bo, outT):
    persist_cm = tc.tile_pool(name="persist", bufs=1)
    persist = persist_cm.__enter__()
    qT_sb = persist.tile([128, DT, S], BF16)
    kT_sb = persist.tile([128, DT, S], BF16)
    v_sb = persist.tile([128, KC, H * VW], FP16)
    ctxT_sb = persist.tile([128, DT, S], BF16)
    bo_sb = persist.tile([128, DT, 1], F32)
    nc.sync.dma_start(out=bo_sb, in_=bo[:].rearrange("(t p) -> p t", p=128))
    nc.vector.memset(
        v_sb[:].rearrange("p a (h c) -> p a h c", c=VW)[:, :, :, HD : HD + 1], 1.0
    )

    xpool_cm = tc.tile_pool(name="xpool", bufs=1)
    xpool = xpool_cm.__enter__()
    xT_sb = xpool.tile([128, DT, S], BF16)
    for t in range(DT):
        # scalar HWDGE queue: runs in parallel with the weight-stripe DMAs
        # on the sync queue, so the first projection isn't DMA-serialized
        nc.scalar.dma_start(out=xT_sb[:, t, :], in_=xT[t * 128 : (t + 1) * 128, :])

    # -------- main loop: per-head-pair projection + attention ----
    # (the v projection is folded into the loop in 2-pair chunks so the
    # PE keeps dense filler while ACT grinds through the exps)
    def _v_chunk(tp):
        """v (token-major) for head pairs tp, tp+1 (4 heads, 256 cols)."""
        stripe = wvpool.tile([128, DT, 256], BF16, tag="wv", name=f"wv{tp}")
        nc.sync.dma_start(
            out=stripe,
            in_=wvT[:, tp * 256 : (tp + 1) * 256].rearrange(
                "(t p) n -> p t n", p=128
            ),
        )
        for st in range(KC):
            vps = pp.tile([128, 256], F32, tag="pp", name=f"vps{tp}_{st}")
            for kk in range(DT):
                nc.tensor.matmul(
                    vps, lhsT=xT_sb[:, kk, st * 128 : (st + 1) * 128],
                    rhs=stripe[:, kk, :],
                    start=(kk == 0), stop=(kk == DT - 1),
                )
            nc.vector.tensor_copy(
                out=v_sb[
                    :, st, (4 * tp) * VW : (4 * tp + 4) * VW
                ].rearrange("p (h c) -> p h c", c=VW)[:, :, 0:HD],
                in_=vps[:].rearrange("p (h c) -> p h c", c=HD),
            )

    with ExitStack() as p2:
        wpool = p2.enter_context(tc.tile_pool(name="wpool", bufs=4))
        wvpool = p2.enter_context(tc.tile_pool(name="wvpool", bufs=2))
        wpool3 = p2.enter_context(tc.tile_pool(name="wpool3", bufs=2))
        epool = p2.enter_context(tc.tile_pool(name="epool", bufs=3))
        small = p2.enter_context(tc.tile_pool(name="small", bufs=2))
        pp = p2.enter_context(tc.tile_pool(name="pp", bufs=2, space="PSUM"))
        sc = p2.enter_context(tc.tile_pool(name="sc", bufs=2, space="PSUM"))
        cx = p2.enter_context(tc.tile_pool(name="cx", bufs=2, space="PSUM"))

        _qk_t(nc, xT_sb, wpool, pp, wqT, qT_sb, 0)
        _qk_t(nc, xT_sb, wpool, pp, wkT, kT_sb, 0)
        _v_chunk(0)

        for t in range(DT):
            if t == 9:
                # pair 9 is projected just-in-time at the top of its own
                # iteration so the final iteration has dense PE filler
                _qk_t(nc, xT_sb, wpool, pp, wqT, qT_sb, 9)
                _qk_t(nc, xT_sb, wpool, pp, wkT, kT_sb, 9)
            exps = [
                epool.tile([128, KC, S], FP16, tag="exp", name=f"exp{t}_{i}")
                for i in range(2)
            ]
            # scoresT + exp; head pair (2t, 2t+1) row-packs the PE: the
            # two halves' matmuls are issued alternately (h0,h1,h0,h1) so
            # the PE runs them concurrently in row groups 0-63 / 64-127
            for kc in range(KC):
                pss = [
                    sc.tile([128, 1024], F32, tag="sc", name=f"scps{half}")
                    for half in range(2)
                ]
                for qc in range(2):
                    for half in range(2):
                        p0 = half * 64
                        nc.tensor.matmul(
                            pss[half][:, qc * 512 : (qc + 1) * 512],
                            lhsT=kT_sb[p0 : p0 + 64, t, kc * 128 : (kc + 1) * 128],
                            rhs=qT_sb[p0 : p0 + 64, t, qc * 512 : (qc + 1) * 512],
                            start=True,
                            stop=True,
                        )
                for half in range(2):
                    nc.scalar.activation(
                        out=exps[half][:, kc, :],
                        in_=pss[half],
                        func=EXP,
                        scale=ATTN_SCALE,
                    )

            # pair t+1's projection one iteration ahead — except pair 9,
            # which is projected just-in-time inside its own iteration so
            # the final iteration still has dense PE filler under ACT.
            if t + 1 < DT and t != 8:
                _qk_t(nc, xT_sb, wpool, pp, wqT, qT_sb, t + 1)
                _qk_t(nc, xT_sb, wpool, pp, wkT, kT_sb, t + 1)
            if t % 2 == 1 and t < 8:
                _v_chunk((t + 1) // 2)
            # AV + sumexp + normalize
            for half in range(2):
                h = 2 * t + half
                stage = small.tile([HD + 1, S], F32, tag="stage", name="stage")
                for qc in range(2):
                    cps = cx.tile([HD + 1, 512], F32, tag="cx", name="cxps")
                    for kc in range(KC):
                        nc.tensor.matmul(
                            cps,
                            lhsT=v_sb[:, kc, h * VW : (h + 1) * VW],
                            rhs=exps[half][:, kc, qc * 512 : (qc + 1) * 512],
                            start=(kc == 0),
                            stop=(kc == KC - 1),
                        )
                    nc.vector.tensor_copy(
                        out=stage[:, qc * 512 : (qc + 1) * 512], in_=cps
                    )
                # 1/sumexp: reshape the [1, S] row across 64 DVE lanes
                r64 = small.tile([64, 16], F32, tag="r64", name="r64")
                nc.sync.dma_start(out=r64, in_=stage[HD : HD + 1, :])
                rc64 = small.tile([64, 16], F32, tag="rc64", name="rc64")
                nc.vector.reciprocal(rc64, r64)
                rrow = small.tile([1, S], F32, tag="rrow", name="rrow", bufs=1)
                nc.sync.dma_start(out=rrow, in_=rc64)
                bcast = small.tile([HD, S], F32, tag="bcast", name="bcast", bufs=1)
                nc.gpsimd.partition_broadcast(bcast, rrow)
                nc.vector.tensor_mul(
                    ctxT_sb[half * 64 : half * 64 + 64, t, :],
                    stage[0:HD, :],
                    bcast,
                )

        # ------------- phase O: output projection (same pool scope:
        # no inter-phase barrier; Wo stripes prefetch on the scalar DMA
        # queue while the attention tail drains; psum shares the pp ring)
        for mg in range(MG):
            stripe3 = wpool3.tile([128, DT, 256], BF16, tag="w3", name=f"w3_{mg}")
            nc.scalar.dma_start(
                out=stripe3,
                in_=woT[:, mg * 256 : (mg + 1) * 256].rearrange(
                    "(t p) n -> p t n", p=128
                ),
            )
            for ml in range(2):
                m = mg * 2 + ml
                for qc in range(2):
                    ops = pp.tile([128, 512], F32, tag="pp", name="opps")
                    for kk in range(DT):
                        nc.tensor.matmul(
                            ops,
                            lhsT=stripe3[:, kk, ml * 128 : (ml + 1) * 128],
                            rhs=ctxT_sb[:, kk, qc * 512 : (qc + 1) * 512],
                            start=(kk == 0),
                            stop=(kk == DT - 1),
                        )
                    o_sb = small.tile(
                        [128, 512], F32, tag="osb", name="osb", bufs=4
                    )
                    nc.vector.tensor_scalar_add(o_sb, ops, bo_sb[:, m, :])
                    nc.sync.dma_start(
                        out=outT[m * 128 : (m + 1) * 128, qc * 512 : (qc + 1) * 512],
                        in_=o_sb,
                    )

    xpool_cm.__exit__(None, None, None)
    persist_cm.__exit__(None, None, None)


def build_nc():
    nc = bacc.Bacc(None, target_bir_lowering=False)
    xT = nc.dram_tensor("xT", [D, S], BF16, kind="ExternalInput")
    wqT = nc.dram_tensor("wqT", [D, D], BF16, kind="ExternalInput")
    wkT = nc.dram_tensor("wkT", [D, D], BF16, kind="ExternalInput")
    wvT = nc.dram_tensor("wvT", [D, D], BF16, kind="ExternalInput")
    woT = nc.dram_tensor("woT", [D, D], BF16, kind="ExternalInput")
    bo = nc.dram_tensor("bo", [D], F32, kind="ExternalInput")
    outT = nc.dram_tensor("outT", [D, S], F32, kind="ExternalOutput")
    with tile.TileContext(nc) as tc:
        _emit(nc, tc, xT, wqT, wkT, wvT, woT, bo, outT)
    nc.compile()
    return nc


_NC = None


def _get_nc():
    global _NC
    if _NC is None:
        _NC = build_nc()
    return _NC


def make_in_maps(hidden_states, Wq, Wk, Wv, Wo, bo, Aq, Bq, Ak, Bk, Av, Bv, Ao, Bo):
    x = np.asarray(hidden_states, dtype=np.float32)

    def eff_T(W, A, Bup):
        W64 = np.asarray(W, dtype=np.float64)
        lora = np.asarray(Bup, dtype=np.float64) @ np.asarray(A, dtype=np.float64)
        return np.ascontiguousarray(
            (W64 + SCALING * lora).T.astype(ml_dtypes.bfloat16)
        )

    base = {
        "wqT": eff_T(Wq, Aq, Bq),
        "wkT": eff_T(Wk, Ak, Bk),
        "wvT": eff_T(Wv, Av, Bv),
        "woT": eff_T(Wo, Ao, Bo),
        "bo": np.ascontiguousarray(np.asarray(bo, dtype=np.float32)),
    }
    return [
        dict(base, xT=np.ascontiguousarray(x[b].T.astype(ml_dtypes.bfloat16)))
        for b in range(x.shape[0])
    ]


def kernel(**inputs):
    in_maps = make_in_maps(**inputs)
    nc = _get_nc()
    res = run_bass_kernel_spmd(nc, in_maps, core_ids=list(range(N_CORES)))
    out = np.stack([res.results[b]["outT"].T for b in range(N_CORES)])
    return np.ascontiguousarray(out, dtype=np.float32)
